# revision 30
# baseline (speedup 1.0000x reference)
"""Multi-head self-attention (B=2, N=4096, D=768, H=12, dh=64) on 8 trn2 NeuronCores.

Sharding: core c handles batch b=c//4 and heads 3*(c%4)..3*(c%4)+2 (head-parallel
attention), then an AllGather redistributes head-outputs so each core projects its
own token quarter with the full w_out (token-parallel output projection).

Per-core pipeline (all matmuls fp32r):
  1. qkv projection from host-transposed xT (own PSUM pool, triple-buffered so
     the m-chunk pipeline never stalls the PE into a HAM cold window).
  2. flash-style attention, query chunks in pairs: row-tiled QK pair -> exp
     (split ACT table exp / custom 1-op DVE polynomial exp16) -> PV with an
     appended ones-column producing the softmax denominator -> normalize.
     h0/h1 norms are deferred past the h2 transition so the PE FIFO never
     stalls behind the reciprocal chain; a post-build IR pass dedups repeated
     LDWEIGHTS and nosync chains pin the PE order.
  3. output projection is interleaved at group boundaries (quarters whose
     AllGather completed a group ago), reusing the freed po1 PSUM bank, so
     boundary bubbles hold PE work and the tail only exposes one collective.
"""
import sys

sys.path.insert(0, "/opt/trn_rl_repo")

import numpy as np

import concourse.bass as bass
import concourse.mybir as mybir
import concourse.tile as tile
import concourse.bacc as bacc
from concourse.masks import make_identity
from concourse.tile_rust import add_dep_helper


def _chain(insts):
    """Pin relative PE order with nosync edges so the Tile scheduler cannot
    interleave other matmuls between stationary-sharing pairs (which would
    clobber the array weight state the LDW dedup pass relies on)."""
    raw = [getattr(x, "ins", x) for x in insts]
    for a, b in zip(raw, raw[1:]):
        add_dep_helper(b, a, False, "pe-order-pin")
    return insts


N_CORES = 8
B, N, D, H, DH = 2, 4096, 768, 12, 64
HPC = 3            # heads per core
NQ = N // 4        # tokens per core quarter (1024)
SCALE = D ** -0.5
F32 = mybir.dt.float32
F32R = mybir.dt.float32r
AF = mybir.ActivationFunctionType
BF16 = mybir.dt.bfloat16
IC = 512           # query chunk (i-chunk) size
NIC = N // IC      # 8 i-chunks
NTAU = N // IC     # token chunks for projection (512 wide)
VW = 195           # v_sb row stride: [v0|1|v1|1|v2|1] = 3*65

# i-chunk groups: pairs share stationaries (LDW dedup), last two single so
# their AllGathers stagger and the tail exposes only one collective.
# ph3[g] = quarters whose output projection is emitted after group g
# (their AllGathers completed at least one full group earlier).
IC_GROUPS = [
    ((0, 1), ()),
    ((2, 3), (0, 1)),
    ((4, 5), (2, 3)),
    ((6,), (4, 5)),
    ((7,), (6,)),
]
TAIL_QUARTERS = (7,)


# ---------------------------------------------------------------- custom DVE exp
def _register_exp_ops():
    """exp(16u) in ONE DVE op: (((u+1)^2 + 1) * 0.5)^16 = taylor2(u)^16.

    taylor2(u) = 1 + u + u^2/2 via ((u+1)^2 + 1)/2; 8 ALU stages exactly:
    add, sq, add, mul, sq, sq, sq, sq.  rel err ~ logit^3/1536 (<6e-3 at
    |logit|<=2), negligible after softmax averaging.
    """
    import concourse.dve_ops as dve_ops
    from concourse.dve_ops import DveOp, OPS, CUSTOM_DVE_SPECS, _SUB_OPCODE_FOR_NAME
    from concourse.dve_spec import Spec, Src0, C0, One, sq, lower
    from concourse.dve_uop import DveOpSpec

    if "EXP16_ANT" in _SUB_OPCODE_FOR_NAME:
        return dve_ops.EXP16_ANT

    u = Src0
    spec_e16 = Spec(
        body=sq(sq(sq(sq((sq(u + One) + One) * C0)))),
        reference=lambda in0, in1, s0, s1, imm2: (
            (((in0 + 1.0) ** 2 + 1.0) * s0) ** 16
        ),
    )

    def _mk(name, spec):
        opcode = max(_SUB_OPCODE_FOR_NAME.values()) + 1
        _SUB_OPCODE_FOR_NAME[name] = opcode
        shas = {}
        for ver in ("v3", "v4"):
            s = DveOpSpec(
                name=name, opcode=opcode, uops=lower(spec, ver=ver), rd1_en=False
            )
            shas[ver] = s.sha(ver)
        op = DveOp(name, spec, subdim=False, uops_sha=shas)
        OPS.append(op)
        CUSTOM_DVE_SPECS[name] = spec
        setattr(dve_ops, name, op)
        return op

    return _mk("EXP16_ANT", spec_e16)


# ------------------------------------------------------- LDWEIGHTS dedup pass
def _dedup_ldweights(nc):
    """Remove InstLdweights whose exact stationary (AP+tile pos/size) is
    already loaded in the same array row-region with no intervening clobber.

    bass emits an Ldweights before every Matmult even when the stationary is
    unchanged.  All stationaries that can match here are read-only during
    their phase (persistent k/v/weight tiles), so an exact AP match means
    identical array contents.  Transposes conservatively clobber all tracked
    state.  Deps of a removed LDW move to its Matmult; references to the
    removed name are remapped there too.
    """
    PE = mybir.EngineType.PE
    total = 0
    for f in nc.m.functions:
        for b in f.blocks:
            il = b.instructions
            pe_idx = [i for i, ins in enumerate(il) if ins.engine == PE]
            regions = []  # [row_lo, row_hi, signature]
            drops = []    # (list_index, dropped_name, target_name)
            for k, i in enumerate(pe_idx):
                ins = il[i]
                tn = type(ins).__name__
                if tn == 'InstLdweights':
                    if ins.is_transpose:
                        regions = []
                        continue
                    tp = tuple(ins.tile_position or (0, 0))
                    ts = tuple(ins.tile_size) if ins.tile_size else (128, 128)
                    lo, hi = tp[0], tp[0] + ts[0]
                    sig = (str(ins.ins[0]), tp, ts, str(ins.perf_mode))
                    hit = any(r[0] == lo and r[1] == hi and r[2] == sig
                              for r in regions)
                    if hit:
                        # next PE instruction must be this LDW's matmul
                        nxt = il[pe_idx[k + 1]] if k + 1 < len(pe_idx) else None
                        if nxt is not None and type(nxt).__name__ == 'InstMatmult':
                            nxt.add_sync_dependencies_from(
                                ins.sync_dependency_set_copy())
                            nxt.add_nosync_dependencies_from(
                                ins.nosync_dependency_set_copy())
                            drops.append((i, ins.name, nxt.name))
                            continue
                    regions = [r for r in regions if r[1] <= lo or r[0] >= hi]
                    regions.append([lo, hi, sig])
                elif tn in ('InstMatmult', 'InstMatmultMx'):
                    if getattr(ins, 'is_transpose', False):
                        regions = []
            if drops:
                name_map = {nm: tgt for _, nm, tgt in drops}
                dropped = set(name_map)
                for ins in il:
                    deps = set(ins.sync_dependency_names()) | set(
                        ins.nosync_dependency_names())
                    if deps & dropped:
                        ins.remap_dependency_names(name_map)
                for i, _, _ in sorted(drops, reverse=True):
                    del il[i]
                total += len(drops)
    return total


# ---------------------------------------------------------------- program build
_PROG_CACHE = {}


def build_program(use_dve_exp=True, use_fast_recip=True, dedup=True):
    key = ("prog", use_dve_exp, use_fast_recip, dedup)
    if key in _PROG_CACHE:
        return _PROG_CACHE[key]
    e16_op = _register_exp_ops()

    nc = bacc.Bacc("TRN2", target_bir_lowering=False, debug=False, num_devices=N_CORES)

    xT = nc.dram_tensor("xT", [D, N], BF16, kind="ExternalInput").ap()
    wqkv = nc.dram_tensor("wqkv", [D, 640], BF16, kind="ExternalInput").ap()
    bqkv = nc.dram_tensor("bqkv", [5, 128], F32, kind="ExternalInput").ap()
    wout = nc.dram_tensor("wout", [D, 192], F32, kind="ExternalInput").ap()
    bout = nc.dram_tensor("bout", [2, 128], F32, kind="ExternalInput").ap()
    y = nc.dram_tensor("y", [HPC * DH, N], F32, kind="ExternalOutput").ap()


    with tile.TileContext(nc, trace_sim=False) as tc:
        with (
            tc.tile_pool(name="consts", bufs=1) as consts,
            tc.tile_pool(name="persist", bufs=1) as persist,
            tc.tile_pool(name="otp", bufs=3) as otp,
            tc.tile_pool(name="dram", bufs=1, space="DRAM") as dram,
            tc.tile_pool(name="ph3", bufs=1) as ph3,
        ):
            # ---------------- constants
            ident_f = consts.tile([128, 128], F32)
            make_identity(nc, ident_f[:])
            ident = consts.tile([128, 128], BF16)
            nc.scalar.copy(ident[:], ident_f[:])

            ones_f = consts.tile([128, 64], F32)
            nc.vector.memset(ones_f[:], 1.0)
            # K=1 stationary for the 1/l broadcast (base partition 0, same
            # as rr and po's l row - custom-DVE lanes are tied to partitions)
            ones1_t = consts.tile([128, 64], F32R)
            nc.scalar.copy(ones1_t[:], ones_f[:])
            ones1 = ones1_t[0:1, :]

            wq_sb = consts.tile([128, 6 * 640], BF16)  # 6 f-chunks of [128,640]
            for fc in range(6):
                # spread the startup loads across DMA-capable queues
                eng = (nc.sync, nc.scalar, nc.gpsimd)[fc % 3]
                eng.dma_start(
                    out=wq_sb[:, fc * 640 : (fc + 1) * 640],
                    in_=wqkv[fc * 128 : (fc + 1) * 128, :],
                )
            bq_sb = consts.tile([128, 5], F32)  # per-m-chunk bias columns
            for m in range(5):
                nc.sync.dma_start(
                    out=bq_sb[:, m : m + 1],
                    in_=bqkv[m : m + 1, :].rearrange("a p -> p a"),
                )

            # phase-3 weights, prefetched at program start on the gpsimd queue
            wo_sb = ph3.tile([128, 6 * 192], BF16)  # w_out col-slice, 6 d-chunks
            for dc in range(6):
                nc.gpsimd.dma_start(
                    out=wo_sb[:, dc * 192 : (dc + 1) * 192],
                    in_=wout[dc * 128 : (dc + 1) * 128, :],
                )
            bo_sb = ph3.tile([128, 2], F32)
            for m in range(2):
                nc.sync.dma_start(
                    out=bo_sb[:, m : m + 1],
                    in_=bout[m : m + 1, :].rearrange("a p -> p a"),
                )

            # ---------------- persistent activations
            qT01 = persist.tile([128, N], BF16)  # rows 0:64 head0 qT, 64:128 head1
            qT2 = persist.tile([128, N], BF16)   # head2 qT duplicated in both halves
            kT01 = persist.tile([128, N], BF16)  # scaled kT, heads 0/1
            kT2 = persist.tile([128, N], BF16)   # scaled kT head2, duplicated
            v_sb = persist.tile([128, 32 * VW], BF16)  # [tok128][v0|1|v1|1|v2|1]/j-tile

            # ones columns of v_sb (positions 64,129,194 of each 195 block)
            ones32 = consts.tile([128, 32], F32)
            nc.vector.memset(ones32[:], 1.0)
            v_view = v_sb[:].rearrange("p (t c) -> p t c", c=VW)
            for col in (64, 129, 194):
                nc.scalar.copy(
                    v_view[:, :, col : col + 1],
                    ones32[:].rearrange("p (a b) -> p a b", b=1),
                )

            # AllGather eighths: in [192, 512] -> out [768, 512] (rank-major rows)
            oT_q = [dram.tile([HPC * DH, IC], BF16, name=f"oT_q{i}") for i in range(8)]
            ag_q = [dram.tile([D, IC], BF16, name=f"ag_q{i}") for i in range(8)]

            with tc.tile_pool(name="work", bufs=2) as work, \
                 tc.tile_pool(name="ptp", bufs=3) as ptp:
                # ---------------- phase 1: qkv projection (own PSUM pool)
                # 5 m-chunks ([v0|v1],[v2|0],[q0|q1],[k0|k1]s,[q2|k2]s) over
                # tau-blocks of 4, stationary (m,fc) reused across the block
                # so the dedup pass strips 3 of every 4 LDWEIGHTS; head-2
                # q/k row duplicates are built by DVE copies instead of
                # recomputing a whole m-chunk.
                with tc.tile_pool(name="ps1", bufs=6, space="PSUM") as ps1:
                    TB = 4
                    for tb in range(NTAU // TB):
                        taus = range(tb * TB, (tb + 1) * TB)
                        xts = {}
                        for fc in range(6):
                            for tau in taus:
                                tsl = slice(tau * IC, (tau + 1) * IC)
                                xt = work.tile([128, IC], BF16, tag="xt",
                                               bufs=26, name=f"xt_{fc}_{tau}")
                                eng = nc.sync if fc % 2 == 0 else nc.scalar
                                eng.dma_start(
                                    out=xt[:],
                                    in_=xT[fc * 128 : (fc + 1) * 128, tsl],
                                )
                                xts[(fc, tau)] = xt
                        vt01s = {tau: work.tile([128, IC], BF16, tag="vt01",
                                                bufs=5, name=f"vt01_{tau}")
                                 for tau in taus}
                        vt2s = {tau: work.tile([128, IC], BF16, tag="vt2",
                                               bufs=5, name=f"vt2_{tau}")
                                for tau in taus}

                        def do_mb(m):
                            pps = {tau: ps1.tile([128, IC], F32, tag="pp",
                                                 bufs=6, name=f"pp_{m}_{tau}")
                                   for tau in taus}
                            mms = []
                            for fc in range(6):
                                for tau in taus:
                                    mms.append(nc.tensor.matmul(
                                        pps[tau][:, 0:IC],
                                        wq_sb[:, fc * 640 + m * 128
                                              : fc * 640 + (m + 1) * 128],
                                        xts[(fc, tau)][:],
                                        start=(fc == 0),
                                        stop=(fc == 5),
                                    ))
                            _chain(mms)
                            bias = bq_sb[:, m : m + 1]
                            for tau in taus:
                                tsl = slice(tau * IC, (tau + 1) * IC)
                                pp = pps[tau]
                                if m == 0:
                                    nc.scalar.activation(
                                        vt01s[tau][:], pp[:, 0:IC],
                                        AF.Identity, bias=bias)
                                elif m == 1:
                                    nc.scalar.activation(
                                        vt2s[tau][:], pp[:, 0:IC],
                                        AF.Identity, bias=bias)
                                elif m == 2:
                                    nc.scalar.activation(
                                        qT01[:, tsl], pp[:, 0:IC],
                                        AF.Identity, bias=bias)
                                elif m == 3:
                                    nc.scalar.activation(
                                        kT01[:, tsl], pp[:, 0:IC],
                                        AF.Identity, bias=bias)
                                else:
                                    # [q2|k2]: split evacuation + DVE dup
                                    nc.scalar.activation(
                                        qT2[0:64, tsl], pp[0:64, 0:IC],
                                        AF.Identity, bias=bias[0:64, :])
                                    nc.scalar.activation(
                                        kT2[64:128, tsl], pp[64:128, 0:IC],
                                        AF.Identity, bias=bias[64:128, :])
                                    # cross-partition-base dup copies (ACT
                                    # handles the base move; DVE is lane-bound)
                                    nc.scalar.copy(
                                        qT2[64:128, tsl], qT2[0:64, tsl])
                                    nc.scalar.copy(
                                        kT2[0:64, tsl], kT2[64:128, tsl])

                        def vtrans(taus=taus, vt01s=vt01s, vt2s=vt2s):
                            for tau in taus:
                                for t in range(4):
                                    jt = 4 * tau + t
                                    base = jt * VW
                                    pv = ps1.tile([128, IC], BF16, tag="pv",
                                                  bufs=2, name=f"pv_{jt}")
                                    nc.tensor.transpose(
                                        pv[:, 0:128],
                                        vt01s[tau][:, t * 128 : (t + 1) * 128],
                                        ident[:]
                                    )
                                    nc.vector.tensor_copy(
                                        v_sb[:, base : base + 64], pv[:, 0:64]
                                    )
                                    nc.vector.tensor_copy(
                                        v_sb[:, base + 65 : base + 129],
                                        pv[:, 64:128]
                                    )
                                    pv2 = ps1.tile([128, IC], BF16, tag="pv",
                                                   bufs=2, name=f"pv2_{jt}")
                                    nc.tensor.transpose(
                                        pv2[:, 0:64],
                                        vt2s[tau][0:64, t * 128 : (t + 1) * 128],
                                        ident[0:64, 0:64],
                                    )
                                    nc.vector.tensor_copy(
                                        v_sb[:, base + 130 : base + 194],
                                        pv2[:, 0:64]
                                    )

                        do_mb(0)
                        do_mb(1)
                        do_mb(2)
                        vtrans()
                        do_mb(3)
                        do_mb(4)

                # ---------------- phase 2: attention (+ interleaved phase 3)
                with tc.tile_pool(name="spsum", bufs=2, space="PSUM") as spsum, \
                     tc.tile_pool(name="opsum", bufs=2, space="PSUM") as opsum:

                    def evac_po(po):
                        """Fast ACT evacuation of a [65,512] po psum (row 64 =
                        l staged to a base-partition-0 row for the custom-DVE
                        reciprocal).  Frees the PSUM bank immediately so the
                        norm finish can happen slots later without stalling
                        the PE FIFO behind the reciprocal chain."""
                        lsb = otp.tile([1, IC], F32, tag="lsb", bufs=6)
                        nc.scalar.copy(lsb[:], po[64:65, :])
                        dat = otp.tile([64, IC], F32, tag="dat", bufs=6)
                        nc.scalar.copy(dat[:], po[0:64, :])
                        return lsb, dat

                    def finish_norm(lsb, dat, ic, h):
                        """1/l + K=1 broadcast matmul + multiply + DMA."""
                        rr_t = otp.tile([1, IC], F32R, tag="rr", bufs=8)
                        rr = rr_t[:]
                        if use_fast_recip:
                            from concourse.dve_ops import (
                                RECIP_APPROX_FAST_CONSTS as _RC,
                                RECIPROCAL_APPROX_FAST as _RF,
                            )
                            with nc.allow_low_precision(reason="fp32r rounding of 1/l"):
                                nc.vector._custom_dve(
                                    _RF, out=rr, in0=lsb[:],
                                    s0=_RC["s0"], s1=_RC["s1"], imm2=_RC["imm2"],
                                )
                        else:
                            with nc.allow_low_precision(reason="fp32r is bit-identical fp32"):
                                nc.vector.reciprocal(rr, lsb[:])
                        pr = spsum.tile([128, 1024], F32, tag="sp", bufs=2)
                        nc.tensor.matmul(
                            pr[0:64, 0:IC], ones1, rr,
                            start=True, stop=True,
                        )
                        rb = otp.tile([64, IC], F32, tag="rb", bufs=6)
                        nc.scalar.copy(rb[:], pr[0:64, 0:IC])
                        ot = otp.tile([64, IC], BF16, tag="ot", bufs=6)
                        nc.vector.tensor_tensor(
                            out=ot[:], in0=dat[:], in1=rb[:],
                            op=mybir.AluOpType.mult,
                        )
                        nc.sync.dma_start(
                            out=oT_q[ic][64 * h : 64 * h + 64, :], in_=ot[:]
                        )

                    def emit_og_loads(quarters, ogs):
                        for qtr in quarters:
                            agr = ag_q[qtr][:]
                            ogs[qtr] = []
                            for dc in range(6):
                                og = ph3.tile([128, IC], BF16, tag="og", bufs=13)
                                nc.gpsimd.dma_start(
                                    out=og[:],
                                    in_=agr[dc * 128 : (dc + 1) * 128, :],
                                )
                                ogs[qtr].append(og)

                    last_ph3 = [None]

                    def emit_ph3(quarters, ogs):
                        """Output projection for quarters whose AllGather is
                        done; py reuses the freed po1 PSUM bank."""
                        if not quarters:
                            return
                        for ec, (elo, ew) in enumerate(((0, 128), (128, 64))):
                            pys = {q: opsum.tile([128, IC], F32, tag="po1",
                                                 bufs=2, name=f"py_{q}_{ec}")
                                   for q in quarters}
                            mms = []
                            for dc in range(6):
                                for q in quarters:
                                    mms.append(nc.tensor.matmul(
                                        pys[q][0:ew, 0:IC],
                                        wo_sb[:, dc * 192 + elo : dc * 192 + elo + ew],
                                        ogs[q][dc][:],
                                        start=(dc == 0), stop=(dc == 5),
                                    ))
                            _chain(mms)
                            last_ph3[0] = mms[-1]
                            for q in quarters:
                                ysb = ph3.tile([128, IC], F32, tag="ysb", bufs=3)
                                nc.scalar.activation(
                                    ysb[0:ew, :], pys[q][0:ew, 0:IC], AF.Identity,
                                    bias=bo_sb[0:ew, ec : ec + 1],
                                )
                                nc.sync.dma_start(
                                    out=y[elo : elo + ew,
                                          q * IC : (q + 1) * IC],
                                    in_=ysb[0:ew, :],
                                )

                    # Software-pipelined slot stream: QK(s)+exp(s) at slot s,
                    # PV(s) deferred so each exp has a full slot period before
                    # its PV reaches the head of the strict-FIFO PE queue.
                    slot_ctr = [0]
                    last_qk = [None]
                    ogs = {}

                    for grp, ph3_quarters in IC_GROUPS:
                        ics = list(grp)
                        ng = len(ics)
                        po0s = [opsum.tile([128, IC], F32, tag="po0", bufs=2,
                                           name=f"po0_{ic}") for ic in ics]
                        po1s = [opsum.tile([128, IC], F32, tag="po1", bufs=2,
                                           name=f"po1_{ic}") for ic in ics]
                        po2_box = [None]
                        h01_norms = [None]
                        pending = []
                        depth = 1 if ng == 2 else 2

                        def push_slot(fn, pending=pending, depth=depth):
                            pending.append(fn)
                            if len(pending) > depth:
                                pending.pop(0)()

                        def exp_batches(sps, ptp=ptp):
                            pts = []
                            for i, sp in enumerate(sps):
                                pt = ptp.tile([128, 1024], BF16, tag="pt", bufs=5)
                                use_dve = (i == 1) if len(sps) == 2 else (
                                    slot_ctr[0] % 2 == 1)
                                if use_dve_exp and use_dve:
                                    nc.vector._custom_dve(
                                        e16_op, out=pt[:], in0=sp[:], s0=0.5)
                                else:
                                    nc.scalar.activation(
                                        pt[:], sp[:], AF.Exp, scale=16.0)
                                pts.append(pt)
                            slot_ctr[0] += 1
                            return pts

                        # --- heads 0,1 (row-tiled pair over the same key tile)
                        for jc in range(32):
                            sps, qks = [], []
                            for ic in ics:
                                isl = slice(ic * IC, (ic + 1) * IC)
                                sp = spsum.tile([128, 1024], F32, tag="sp", bufs=2)
                                qks.append(nc.tensor.matmul(
                                    sp[:, 0:512],
                                    kT01[0:64, jc * 128 : (jc + 1) * 128],
                                    qT01[0:64, isl],
                                    start=True, stop=True, tile_position=(0, 0),
                                ))
                                qks.append(nc.tensor.matmul(
                                    sp[:, 512:1024],
                                    kT01[64:128, jc * 128 : (jc + 1) * 128],
                                    qT01[64:128, isl],
                                    start=True, stop=True, tile_position=(64, 0),
                                ))
                                sps.append(sp)
                            _chain(qks)
                            last_qk[0] = qks[-1]
                            pts = exp_batches(sps)

                            def pv_h01(pts=pts, jc=jc, po0s=po0s, po1s=po1s,
                                       ics=ics, h01_norms=h01_norms):
                                vbase = jc * VW
                                pvs = []
                                for i in range(len(ics)):
                                    pvs.append(nc.tensor.matmul(
                                        po0s[i][0:65, :],
                                        v_sb[:, vbase : vbase + 65],
                                        pts[i][:, 0:512],
                                        start=(jc == 0), stop=(jc == 31),
                                    ))
                                for i in range(len(ics)):
                                    pvs.append(nc.tensor.matmul(
                                        po1s[i][0:65, :],
                                        v_sb[:, vbase + 65 : vbase + 130],
                                        pts[i][:, 512:1024],
                                        start=(jc == 0), stop=(jc == 31),
                                    ))
                                if last_qk[0] is not None:
                                    _chain([last_qk[0]] + pvs)
                                else:
                                    _chain(pvs)
                                if jc == 31:
                                    # evacuate po psum fast (frees banks for
                                    # po2); the norm finish is emitted slots
                                    # later so the PE never stalls behind the
                                    # reciprocal chain
                                    evs = []
                                    for i, ic in enumerate(ics):
                                        evs.append(evac_po(po0s[i]) + (ic, 0))
                                    for i, ic in enumerate(ics):
                                        evs.append(evac_po(po1s[i]) + (ic, 1))
                                    h01_norms[0] = evs

                            push_slot(pv_h01)

                        # og loads for the boundary ph3 (AllGathers long done)
                        emit_og_loads(ph3_quarters, ogs)

                        # --- head 2 (row-tiled pair over adjacent key tiles)
                        for t in range(16):
                            # finish deferred h0/h1 norms a few slots into h2,
                            # one per slot so the broadcast + psum-slot cycle
                            # slides into the stream without stalling it
                            if h01_norms[0] and t >= 4:
                                finish_norm(*h01_norms[0].pop(0))
                                if not h01_norms[0]:
                                    h01_norms[0] = None
                            sps, qks = [], []
                            for ic in ics:
                                isl = slice(ic * IC, (ic + 1) * IC)
                                sp = spsum.tile([128, 1024], F32, tag="sp", bufs=2)
                                qks.append(nc.tensor.matmul(
                                    sp[:, 0:512],
                                    kT2[0:64, (2 * t) * 128 : (2 * t + 1) * 128],
                                    qT2[0:64, isl],
                                    start=True, stop=True, tile_position=(0, 0),
                                ))
                                qks.append(nc.tensor.matmul(
                                    sp[:, 512:1024],
                                    kT2[64:128, (2 * t + 1) * 128 : (2 * t + 2) * 128],
                                    qT2[64:128, isl],
                                    start=True, stop=True, tile_position=(64, 0),
                                ))
                                sps.append(sp)
                            _chain(qks)
                            last_qk[0] = qks[-1]
                            pts = exp_batches(sps)

                            def pv_h2(pts=pts, t=t, ics=ics, po2_box=po2_box):
                                # allocated lazily so the buffer handoff from
                                # po0 (shared tag) happens after po0's norm
                                if po2_box[0] is None:
                                    po2_box[0] = [
                                        opsum.tile([128, IC], F32, tag="po0",
                                                   bufs=2, name=f"po2_{ic}")
                                        for ic in ics
                                    ]
                                po2s = po2_box[0]
                                pvs = []
                                for s in range(2):
                                    jc = 2 * t + s
                                    vbase = jc * VW
                                    for i in range(len(ics)):
                                        pvs.append(nc.tensor.matmul(
                                            po2s[i][0:65, :],
                                            v_sb[:, vbase + 130 : vbase + 195],
                                            pts[i][:, s * 512 : (s + 1) * 512],
                                            start=(jc == 0), stop=(jc == 31),
                                        ))
                                if last_qk[0] is not None:
                                    _chain([last_qk[0]] + pvs)
                                else:
                                    _chain(pvs)
                                if t == 15:
                                    # fast evacuation only; finish after the
                                    # boundary ph3 is in the FIFO
                                    po2_box.append([
                                        evac_po(po2s[i]) + (ic, 2)
                                        for i, ic in enumerate(ics)
                                    ])

                            push_slot(pv_h2)
                        while pending:
                            pending.pop(0)()

                        # finish h2 norms and fire the collectives first (the
                        # boundary ph3 matmuls below give the scheduler PE
                        # work to cover the reciprocal-chain latency)
                        for lsb, dat, ic, h in po2_box[1]:
                            finish_norm(lsb, dat, ic, h)
                            nc.gpsimd.collective_compute(
                                "AllGather",
                                mybir.AluOpType.bypass,
                                replica_groups=[[0, 1, 2, 3], [4, 5, 6, 7]],
                                ins=[oT_q[ic][:]],
                                outs=[ag_q[ic][:]],
                            )

                        # boundary output projection fills the group-boundary
                        # PE bubble and keeps HAM warm
                        emit_ph3(ph3_quarters, ogs)

                    # tail: warm-keeper matmuls bridge the final AllGather
                    # wait so the last projection doesn't run at HAM half
                    # clock, then the last quarter projects
                    emit_og_loads(TAIL_QUARTERS, ogs)
                    crow_f = consts.tile([128, IC], F32, name="crow_f")
                    nc.vector.memset(crow_f[:], 1.0)
                    crow_t = consts.tile([1, IC], F32R, name="crow")
                    nc.scalar.copy(crow_t[:], crow_f[0:1, :])
                    jnk = opsum.tile([128, IC], F32, tag="po0", bufs=2,
                                     name="junk")
                    warm = []
                    for _ in range(56):
                        warm.append(nc.tensor.matmul(
                            jnk[0:64, 0:IC], ones1, crow_t[:],
                            start=True, stop=True,
                        ))
                    if last_ph3[0] is not None:
                        _chain([last_ph3[0]] + warm)
                    else:
                        _chain(warm)
                    emit_ph3(TAIL_QUARTERS, ogs)

    if dedup:
        removed = _dedup_ldweights(nc)
        print(f"[kernel] dedup_ldweights removed {removed} instructions")
    nc.compile()
    _PROG_CACHE[key] = nc
    return nc


# ---------------------------------------------------------------- host wrapper
def make_in_maps(x, w_qkv, b_qkv, w_out, b_out):
    """Build the 8 per-core input dicts from full inputs."""
    in_maps = []
    import ml_dtypes
    bf16 = ml_dtypes.bfloat16
    xTb = [np.ascontiguousarray(x[b].T.astype(bf16)) for b in range(B)]  # [768, 4096]
    kscale = np.float32(SCALE / 16.0)
    for c in range(N_CORES):
        b = c // 4
        hs = HPC * (c % 4)

        def sect(kind, h):  # q=0,k=1,v=2
            lo = kind * (H * DH) + h * DH
            return w_qkv[:, lo : lo + DH], b_qkv[lo : lo + DH]

        q0, bq0 = sect(0, hs); q1, bq1 = sect(0, hs + 1); q2, bq2 = sect(0, hs + 2)
        k0, bk0 = sect(1, hs); k1, bk1 = sect(1, hs + 1); k2, bk2 = sect(1, hs + 2)
        v0, bv0 = sect(2, hs); v1, bv1 = sect(2, hs + 1); v2, bv2 = sect(2, hs + 2)
        z = np.zeros_like(q2); bz = np.zeros_like(bq2)
        # m-chunks: [v0|v1], [v2|0], [q0|q1], [k0|k1]*s, [q2|k2]*s
        cols = np.concatenate(
            [v0, v1, v2, z, q0, q1, k0 * kscale, k1 * kscale, q2, k2 * kscale],
            axis=1).astype(np.float32)
        bias = np.concatenate(
            [bv0, bv1, bv2, bz, bq0, bq1, bk0 * kscale, bk1 * kscale,
             bq2, bk2 * kscale]).astype(np.float32)
        q = c % 4
        bo = np.zeros((2, 128), np.float32)
        bo[0, :] = b_out[192 * q : 192 * q + 128]
        bo[1, :64] = b_out[192 * q + 128 : 192 * q + 192]
        in_maps.append({
            "xT": xTb[b],
            "wqkv": np.ascontiguousarray(cols.astype(bf16)),
            "bqkv": np.ascontiguousarray(bias.reshape(5, 128)),
            "wout": np.ascontiguousarray(
                w_out[:, 192 * q : 192 * (q + 1)].astype(np.float32)),
            "bout": bo,
        })
    return in_maps


def assemble_output(results):
    out = np.empty((B, N, D), dtype=np.float32)
    for c in range(N_CORES):
        b = c // 4
        q = c % 4
        out[b, :, 192 * q : 192 * (q + 1)] = results[c]["y"].T
    return out


def kernel(x, w_qkv, b_qkv, w_out, b_out):
    from concourse.bass_utils import run_bass_kernel_spmd

    x = np.asarray(x, dtype=np.float32)
    nc = build_program()
    in_maps = make_in_maps(
        x, np.asarray(w_qkv, np.float32), np.asarray(b_qkv, np.float32),
        np.asarray(w_out, np.float32), np.asarray(b_out, np.float32))
    res = run_bass_kernel_spmd(nc, in_maps, core_ids=list(range(N_CORES)))
    return assemble_output(res.results)


# revision 32
# speedup vs baseline: 1.0636x; 1.0636x over previous
"""Multi-head self-attention (B=2, N=4096, D=768, H=12, dh=64) on 8 trn2 NeuronCores.

Sharding: core c handles batch b=c//4 and heads 3*(c%4)..3*(c%4)+2 (head-parallel
attention), then an AllGather redistributes head-outputs so each core projects its
own token quarter with the full w_out (token-parallel output projection).

Per-core pipeline (all matmuls fp32r):
  1. qkv projection from host-transposed xT (own PSUM pool, triple-buffered so
     the m-chunk pipeline never stalls the PE into a HAM cold window).
  2. flash-style attention, query chunks in pairs: row-tiled QK pair -> exp
     (split ACT table exp / custom 1-op DVE polynomial exp16) -> PV with an
     appended ones-column producing the softmax denominator -> normalize.
     h0/h1 norms are deferred past the h2 transition so the PE FIFO never
     stalls behind the reciprocal chain; a post-build IR pass dedups repeated
     LDWEIGHTS and nosync chains pin the PE order.
  3. output projection is interleaved at group boundaries (quarters whose
     AllGather completed a group ago), reusing the freed po1 PSUM bank, so
     boundary bubbles hold PE work and the tail only exposes one collective.
"""
import sys

sys.path.insert(0, "/opt/trn_rl_repo")

import numpy as np

import concourse.bass as bass
import concourse.mybir as mybir
import concourse.tile as tile
import concourse.bacc as bacc
from concourse.masks import make_identity
from concourse.tile_rust import add_dep_helper


def _chain(insts):
    """Pin relative PE order with nosync edges so the Tile scheduler cannot
    interleave other matmuls between stationary-sharing pairs (which would
    clobber the array weight state the LDW dedup pass relies on)."""
    raw = [getattr(x, "ins", x) for x in insts]
    for a, b in zip(raw, raw[1:]):
        add_dep_helper(b, a, False, "pe-order-pin")
    return insts


N_CORES = 8
B, N, D, H, DH = 2, 4096, 768, 12, 64
HPC = 3            # heads per core
NQ = N // 4        # tokens per core quarter (1024)
SCALE = D ** -0.5
F32 = mybir.dt.float32
F32R = mybir.dt.float32r
AF = mybir.ActivationFunctionType
BF16 = mybir.dt.bfloat16
IC = 512           # query chunk (i-chunk) size
NIC = N // IC      # 8 i-chunks
NTAU = N // IC     # token chunks for projection (512 wide)
VW = 195           # v_sb row stride: [v0|1|v1|1|v2|1] = 3*65

# i-chunk groups: pairs share stationaries (LDW dedup), last two single so
# their AllGathers stagger and the tail exposes only one collective.
# ph3[g] = quarters whose output projection is emitted after group g
# (their AllGathers completed at least one full group earlier).
IC_GROUPS = [
    ((0, 1), ()),
    ((2, 3), (0, 1)),
    ((4, 5), (2, 3)),
    ((6,), (4, 5)),
    ((7,), (6,)),
]
TAIL_QUARTERS = (7,)


# ---------------------------------------------------------------- custom DVE exp
def _register_exp_ops():
    """exp(16u) in ONE DVE op: (((u+1)^2 + 1) * 0.5)^16 = taylor2(u)^16.

    taylor2(u) = 1 + u + u^2/2 via ((u+1)^2 + 1)/2; 8 ALU stages exactly:
    add, sq, add, mul, sq, sq, sq, sq.  rel err ~ logit^3/1536 (<6e-3 at
    |logit|<=2), negligible after softmax averaging.
    """
    import concourse.dve_ops as dve_ops
    from concourse.dve_ops import DveOp, OPS, CUSTOM_DVE_SPECS, _SUB_OPCODE_FOR_NAME
    from concourse.dve_spec import Spec, Src0, C0, One, sq, lower
    from concourse.dve_uop import DveOpSpec

    if "EXP16_ANT" in _SUB_OPCODE_FOR_NAME:
        return dve_ops.EXP16_ANT

    u = Src0
    spec_e16 = Spec(
        body=sq(sq(sq(sq((sq(u + One) + One) * C0)))),
        reference=lambda in0, in1, s0, s1, imm2: (
            (((in0 + 1.0) ** 2 + 1.0) * s0) ** 16
        ),
    )

    def _mk(name, spec):
        opcode = max(_SUB_OPCODE_FOR_NAME.values()) + 1
        _SUB_OPCODE_FOR_NAME[name] = opcode
        shas = {}
        for ver in ("v3", "v4"):
            s = DveOpSpec(
                name=name, opcode=opcode, uops=lower(spec, ver=ver), rd1_en=False
            )
            shas[ver] = s.sha(ver)
        op = DveOp(name, spec, subdim=False, uops_sha=shas)
        OPS.append(op)
        CUSTOM_DVE_SPECS[name] = spec
        setattr(dve_ops, name, op)
        return op

    return _mk("EXP16_ANT", spec_e16)


# ------------------------------------------------------- LDWEIGHTS dedup pass
def _dedup_ldweights(nc):
    """Remove InstLdweights whose exact stationary (AP+tile pos/size) is
    already loaded in the same array row-region with no intervening clobber.

    bass emits an Ldweights before every Matmult even when the stationary is
    unchanged.  All stationaries that can match here are read-only during
    their phase (persistent k/v/weight tiles), so an exact AP match means
    identical array contents.  Transposes conservatively clobber all tracked
    state.  Deps of a removed LDW move to its Matmult; references to the
    removed name are remapped there too.
    """
    PE = mybir.EngineType.PE
    total = 0
    for f in nc.m.functions:
        for b in f.blocks:
            il = b.instructions
            pe_idx = [i for i, ins in enumerate(il) if ins.engine == PE]
            regions = []  # [row_lo, row_hi, signature]
            drops = []    # (list_index, dropped_name, target_name)
            for k, i in enumerate(pe_idx):
                ins = il[i]
                tn = type(ins).__name__
                if tn == 'InstLdweights':
                    if ins.is_transpose:
                        regions = []
                        continue
                    tp = tuple(ins.tile_position or (0, 0))
                    ts = tuple(ins.tile_size) if ins.tile_size else (128, 128)
                    lo, hi = tp[0], tp[0] + ts[0]
                    sig = (str(ins.ins[0]), tp, ts, str(ins.perf_mode))
                    hit = any(r[0] == lo and r[1] == hi and r[2] == sig
                              for r in regions)
                    if hit:
                        # next PE instruction must be this LDW's matmul
                        nxt = il[pe_idx[k + 1]] if k + 1 < len(pe_idx) else None
                        if nxt is not None and type(nxt).__name__ == 'InstMatmult':
                            nxt.add_sync_dependencies_from(
                                ins.sync_dependency_set_copy())
                            nxt.add_nosync_dependencies_from(
                                ins.nosync_dependency_set_copy())
                            drops.append((i, ins.name, nxt.name))
                            continue
                    regions = [r for r in regions if r[1] <= lo or r[0] >= hi]
                    regions.append([lo, hi, sig])
                elif tn in ('InstMatmult', 'InstMatmultMx'):
                    if getattr(ins, 'is_transpose', False):
                        regions = []
            if drops:
                name_map = {nm: tgt for _, nm, tgt in drops}
                dropped = set(name_map)
                for ins in il:
                    deps = set(ins.sync_dependency_names()) | set(
                        ins.nosync_dependency_names())
                    if deps & dropped:
                        ins.remap_dependency_names(name_map)
                for i, _, _ in sorted(drops, reverse=True):
                    del il[i]
                total += len(drops)
    return total


# ---------------------------------------------------------------- program build
_PROG_CACHE = {}


def build_program(use_dve_exp=True, use_fast_recip=True, dedup=True):
    key = ("prog", use_dve_exp, use_fast_recip, dedup)
    if key in _PROG_CACHE:
        return _PROG_CACHE[key]
    e16_op = _register_exp_ops()

    nc = bacc.Bacc("TRN2", target_bir_lowering=False, debug=False, num_devices=N_CORES)

    xT = nc.dram_tensor("xT", [D, N], BF16, kind="ExternalInput").ap()
    wqkv = nc.dram_tensor("wqkv", [D, 640], BF16, kind="ExternalInput").ap()
    bqkv = nc.dram_tensor("bqkv", [5, 128], F32, kind="ExternalInput").ap()
    wout = nc.dram_tensor("wout", [D, 192], F32, kind="ExternalInput").ap()
    bout = nc.dram_tensor("bout", [2, 128], F32, kind="ExternalInput").ap()
    y = nc.dram_tensor("y", [HPC * DH, N], F32, kind="ExternalOutput").ap()


    with tile.TileContext(nc, trace_sim=False) as tc:
        with (
            tc.tile_pool(name="consts", bufs=1) as consts,
            tc.tile_pool(name="persist", bufs=1) as persist,
            tc.tile_pool(name="otp", bufs=3) as otp,
            tc.tile_pool(name="dram", bufs=1, space="DRAM") as dram,
            tc.tile_pool(name="ph3", bufs=1) as ph3,
        ):
            # ---------------- constants
            ident_f = consts.tile([128, 128], F32)
            make_identity(nc, ident_f[:])
            ident = consts.tile([128, 128], BF16)
            nc.scalar.copy(ident[:], ident_f[:])

            ones_f = consts.tile([128, 64], F32)
            nc.vector.memset(ones_f[:], 1.0)
            # K=1 stationary for the 1/l broadcast (base partition 0, same
            # as rr and po's l row - custom-DVE lanes are tied to partitions)
            ones1_t = consts.tile([128, 64], F32R)
            nc.scalar.copy(ones1_t[:], ones_f[:])
            ones1 = ones1_t[0:1, :]

            wq_sb = consts.tile([128, 6 * 640], BF16)  # 6 f-chunks of [128,640]
            for fc in range(6):
                # spread the startup loads across DMA-capable queues
                eng = (nc.sync, nc.scalar, nc.gpsimd)[fc % 3]
                eng.dma_start(
                    out=wq_sb[:, fc * 640 : (fc + 1) * 640],
                    in_=wqkv[fc * 128 : (fc + 1) * 128, :],
                )
            bq_sb = consts.tile([128, 5], F32)  # per-m-chunk bias columns
            for m in range(5):
                nc.sync.dma_start(
                    out=bq_sb[:, m : m + 1],
                    in_=bqkv[m : m + 1, :].rearrange("a p -> p a"),
                )

            # phase-3 weights, prefetched at program start on the gpsimd queue
            wo_sb = ph3.tile([128, 6 * 192], BF16)  # w_out col-slice, 6 d-chunks
            for dc in range(6):
                nc.gpsimd.dma_start(
                    out=wo_sb[:, dc * 192 : (dc + 1) * 192],
                    in_=wout[dc * 128 : (dc + 1) * 128, :],
                )
            bo_sb = ph3.tile([128, 2], F32)
            for m in range(2):
                nc.sync.dma_start(
                    out=bo_sb[:, m : m + 1],
                    in_=bout[m : m + 1, :].rearrange("a p -> p a"),
                )

            # ---------------- persistent activations
            qT01 = persist.tile([128, N], BF16)  # rows 0:64 head0 qT, 64:128 head1
            qT2 = persist.tile([128, N], BF16)   # head2 qT duplicated in both halves
            kT01 = persist.tile([128, N], BF16)  # scaled kT, heads 0/1
            kT2 = persist.tile([128, N], BF16)   # scaled kT head2, duplicated
            v_sb = persist.tile([128, 32 * VW], BF16)  # [tok128][v0|1|v1|1|v2|1]/j-tile

            # ones columns of v_sb (positions 64,129,194 of each 195 block)
            ones32 = consts.tile([128, 32], F32)
            nc.vector.memset(ones32[:], 1.0)
            v_view = v_sb[:].rearrange("p (t c) -> p t c", c=VW)
            for col in (64, 129, 194):
                nc.scalar.copy(
                    v_view[:, :, col : col + 1],
                    ones32[:].rearrange("p (a b) -> p a b", b=1),
                )

            # AllGather eighths: in [192, 512] -> out [768, 512] (rank-major rows)
            oT_q = [dram.tile([HPC * DH, IC], BF16, name=f"oT_q{i}") for i in range(8)]
            ag_q = [dram.tile([D, IC], BF16, name=f"ag_q{i}") for i in range(8)]

            with tc.tile_pool(name="work", bufs=2) as work, \
                 tc.tile_pool(name="ptp", bufs=3) as ptp:
                # ---------------- phase 1: qkv projection (own PSUM pool)
                # 5 m-chunks ([v0|v1],[v2|0],[q0|q1],[k0|k1]s,[q2|k2]s) over
                # tau-blocks of 4, stationary (m,fc) reused across the block
                # so the dedup pass strips 3 of every 4 LDWEIGHTS; head-2
                # q/k row duplicates are built by DVE copies instead of
                # recomputing a whole m-chunk.
                with tc.tile_pool(name="ps1", bufs=6, space="PSUM") as ps1:
                    TB = 4
                    for tb in range(NTAU // TB):
                        taus = range(tb * TB, (tb + 1) * TB)
                        xts = {}
                        for fc in range(6):
                            for tau in taus:
                                tsl = slice(tau * IC, (tau + 1) * IC)
                                xt = work.tile([128, IC], BF16, tag="xt",
                                               bufs=26, name=f"xt_{fc}_{tau}")
                                # gpsimd is idle in phase 1; scalar would
                                # block the ACT evacuations behind DMAs
                                eng = nc.sync if fc % 2 == 0 else nc.gpsimd
                                eng.dma_start(
                                    out=xt[:],
                                    in_=xT[fc * 128 : (fc + 1) * 128, tsl],
                                )
                                xts[(fc, tau)] = xt
                        vt01s = {tau: work.tile([128, IC], BF16, tag="vt01",
                                                bufs=5, name=f"vt01_{tau}")
                                 for tau in taus}
                        vt2s = {tau: work.tile([128, IC], BF16, tag="vt2",
                                               bufs=5, name=f"vt2_{tau}")
                                for tau in taus}

                        def do_mb(m):
                            pps = {tau: ps1.tile([128, IC], F32, tag="pp",
                                                 bufs=6, name=f"pp_{m}_{tau}")
                                   for tau in taus}
                            mms = []
                            for fc in range(6):
                                for tau in taus:
                                    mms.append(nc.tensor.matmul(
                                        pps[tau][:, 0:IC],
                                        wq_sb[:, fc * 640 + m * 128
                                              : fc * 640 + (m + 1) * 128],
                                        xts[(fc, tau)][:],
                                        start=(fc == 0),
                                        stop=(fc == 5),
                                    ))
                            _chain(mms)
                            bias = bq_sb[:, m : m + 1]
                            for tau in taus:
                                tsl = slice(tau * IC, (tau + 1) * IC)
                                pp = pps[tau]
                                if m == 0:
                                    nc.scalar.activation(
                                        vt01s[tau][:], pp[:, 0:IC],
                                        AF.Identity, bias=bias)
                                elif m == 1:
                                    nc.scalar.activation(
                                        vt2s[tau][:], pp[:, 0:IC],
                                        AF.Identity, bias=bias)
                                elif m == 2:
                                    nc.scalar.activation(
                                        qT01[:, tsl], pp[:, 0:IC],
                                        AF.Identity, bias=bias)
                                elif m == 3:
                                    nc.scalar.activation(
                                        kT01[:, tsl], pp[:, 0:IC],
                                        AF.Identity, bias=bias)
                                else:
                                    # [q2|k2]: split evacuation + DVE dup
                                    nc.scalar.activation(
                                        qT2[0:64, tsl], pp[0:64, 0:IC],
                                        AF.Identity, bias=bias[0:64, :])
                                    nc.scalar.activation(
                                        kT2[64:128, tsl], pp[64:128, 0:IC],
                                        AF.Identity, bias=bias[64:128, :])
                                    # cross-partition-base dup copies (ACT
                                    # handles the base move; DVE is lane-bound)
                                    nc.scalar.copy(
                                        qT2[64:128, tsl], qT2[0:64, tsl])
                                    nc.scalar.copy(
                                        kT2[0:64, tsl], kT2[64:128, tsl])

                        def vtrans(taus=taus, vt01s=vt01s, vt2s=vt2s):
                            for tau in taus:
                                for t in range(4):
                                    jt = 4 * tau + t
                                    base = jt * VW
                                    pv = ps1.tile([128, IC], BF16, tag="pv",
                                                  bufs=2, name=f"pv_{jt}")
                                    nc.tensor.transpose(
                                        pv[:, 0:128],
                                        vt01s[tau][:, t * 128 : (t + 1) * 128],
                                        ident[:]
                                    )
                                    nc.vector.tensor_copy(
                                        v_sb[:, base : base + 64], pv[:, 0:64]
                                    )
                                    nc.vector.tensor_copy(
                                        v_sb[:, base + 65 : base + 129],
                                        pv[:, 64:128]
                                    )
                                    pv2 = ps1.tile([128, IC], BF16, tag="pv",
                                                   bufs=2, name=f"pv2_{jt}")
                                    nc.tensor.transpose(
                                        pv2[:, 0:64],
                                        vt2s[tau][0:64, t * 128 : (t + 1) * 128],
                                        ident[0:64, 0:64],
                                    )
                                    nc.vector.tensor_copy(
                                        v_sb[:, base + 130 : base + 194],
                                        pv2[:, 0:64]
                                    )

                        do_mb(0)
                        do_mb(1)
                        do_mb(2)
                        vtrans()
                        do_mb(3)
                        do_mb(4)

                # ---------------- phase 2: attention (+ interleaved phase 3)
                with tc.tile_pool(name="spsum", bufs=2, space="PSUM") as spsum, \
                     tc.tile_pool(name="opsum", bufs=2, space="PSUM") as opsum:

                    def evac_po(po):
                        """Fast ACT evacuation of a [65,512] po psum (row 64 =
                        l staged to a base-partition-0 row for the custom-DVE
                        reciprocal).  Frees the PSUM bank immediately so the
                        norm finish can happen slots later without stalling
                        the PE FIFO behind the reciprocal chain."""
                        lsb = otp.tile([1, IC], F32, tag="lsb", bufs=6)
                        nc.scalar.copy(lsb[:], po[64:65, :])
                        dat = otp.tile([64, IC], F32, tag="dat", bufs=6)
                        nc.scalar.copy(dat[:], po[0:64, :])
                        return lsb, dat

                    def finish_norm(lsb, dat, ic, h):
                        """1/l + K=1 broadcast matmul + multiply + DMA."""
                        rr_t = otp.tile([1, IC], F32R, tag="rr", bufs=8)
                        rr = rr_t[:]
                        if use_fast_recip:
                            from concourse.dve_ops import (
                                RECIP_APPROX_FAST_CONSTS as _RC,
                                RECIPROCAL_APPROX_FAST as _RF,
                            )
                            with nc.allow_low_precision(reason="fp32r rounding of 1/l"):
                                nc.vector._custom_dve(
                                    _RF, out=rr, in0=lsb[:],
                                    s0=_RC["s0"], s1=_RC["s1"], imm2=_RC["imm2"],
                                )
                        else:
                            with nc.allow_low_precision(reason="fp32r is bit-identical fp32"):
                                nc.vector.reciprocal(rr, lsb[:])
                        pr = spsum.tile([128, 1024], F32, tag="sp", bufs=2)
                        nc.tensor.matmul(
                            pr[0:64, 0:IC], ones1, rr,
                            start=True, stop=True,
                        )
                        rb = otp.tile([64, IC], F32, tag="rb", bufs=6)
                        nc.scalar.copy(rb[:], pr[0:64, 0:IC])
                        ot = otp.tile([64, IC], BF16, tag="ot", bufs=6)
                        nc.vector.tensor_tensor(
                            out=ot[:], in0=dat[:], in1=rb[:],
                            op=mybir.AluOpType.mult,
                        )
                        nc.sync.dma_start(
                            out=oT_q[ic][64 * h : 64 * h + 64, :], in_=ot[:]
                        )

                    def emit_og_loads(quarters, ogs):
                        for qtr in quarters:
                            agr = ag_q[qtr][:]
                            ogs[qtr] = []
                            for dc in range(6):
                                og = ph3.tile([128, IC], BF16, tag="og", bufs=13)
                                nc.gpsimd.dma_start(
                                    out=og[:],
                                    in_=agr[dc * 128 : (dc + 1) * 128, :],
                                )
                                ogs[qtr].append(og)

                    last_ph3 = [None]

                    def emit_ph3(quarters, ogs):
                        """Output projection for quarters whose AllGather is
                        done; py reuses the freed po1 PSUM bank."""
                        if not quarters:
                            return
                        for ec, (elo, ew) in enumerate(((0, 128), (128, 64))):
                            pys = {q: opsum.tile([128, IC], F32, tag="po1",
                                                 bufs=2, name=f"py_{q}_{ec}")
                                   for q in quarters}
                            mms = []
                            for dc in range(6):
                                for q in quarters:
                                    mms.append(nc.tensor.matmul(
                                        pys[q][0:ew, 0:IC],
                                        wo_sb[:, dc * 192 + elo : dc * 192 + elo + ew],
                                        ogs[q][dc][:],
                                        start=(dc == 0), stop=(dc == 5),
                                    ))
                            _chain(mms)
                            last_ph3[0] = mms[-1]
                            for q in quarters:
                                ysb = ph3.tile([128, IC], F32, tag="ysb", bufs=3)
                                nc.scalar.activation(
                                    ysb[0:ew, :], pys[q][0:ew, 0:IC], AF.Identity,
                                    bias=bo_sb[0:ew, ec : ec + 1],
                                )
                                nc.sync.dma_start(
                                    out=y[elo : elo + ew,
                                          q * IC : (q + 1) * IC],
                                    in_=ysb[0:ew, :],
                                )

                    # Software-pipelined slot stream: QK(s)+exp(s) at slot s,
                    # PV(s) deferred so each exp has a full slot period before
                    # its PV reaches the head of the strict-FIFO PE queue.
                    slot_ctr = [0]
                    last_qk = [None]
                    ogs = {}

                    for grp, ph3_quarters in IC_GROUPS:
                        ics = list(grp)
                        ng = len(ics)
                        po0s = [opsum.tile([128, IC], F32, tag="po0", bufs=2,
                                           name=f"po0_{ic}") for ic in ics]
                        po1s = [opsum.tile([128, IC], F32, tag="po1", bufs=2,
                                           name=f"po1_{ic}") for ic in ics]
                        po2_box = [None]
                        h01_norms = [None]
                        pending = []
                        depth = 1 if ng == 2 else 2

                        def push_slot(fn, pending=pending, depth=depth):
                            pending.append(fn)
                            if len(pending) > depth:
                                pending.pop(0)()

                        def exp_batches(sps, ptp=ptp):
                            pts = []
                            for i, sp in enumerate(sps):
                                pt = ptp.tile([128, 1024], BF16, tag="pt", bufs=5)
                                use_dve = (i == 1) if len(sps) == 2 else (
                                    slot_ctr[0] % 2 == 1)
                                if use_dve_exp and use_dve:
                                    nc.vector._custom_dve(
                                        e16_op, out=pt[:], in0=sp[:], s0=0.5)
                                else:
                                    nc.scalar.activation(
                                        pt[:], sp[:], AF.Exp, scale=16.0)
                                pts.append(pt)
                            slot_ctr[0] += 1
                            return pts

                        # --- heads 0,1 (row-tiled pair over the same key tile)
                        for jc in range(32):
                            sps, qks = [], []
                            for ic in ics:
                                isl = slice(ic * IC, (ic + 1) * IC)
                                sp = spsum.tile([128, 1024], F32, tag="sp", bufs=2)
                                qks.append(nc.tensor.matmul(
                                    sp[:, 0:512],
                                    kT01[0:64, jc * 128 : (jc + 1) * 128],
                                    qT01[0:64, isl],
                                    start=True, stop=True, tile_position=(0, 0),
                                ))
                                qks.append(nc.tensor.matmul(
                                    sp[:, 512:1024],
                                    kT01[64:128, jc * 128 : (jc + 1) * 128],
                                    qT01[64:128, isl],
                                    start=True, stop=True, tile_position=(64, 0),
                                ))
                                sps.append(sp)
                            _chain(qks)
                            last_qk[0] = qks[-1]
                            pts = exp_batches(sps)

                            def pv_h01(pts=pts, jc=jc, po0s=po0s, po1s=po1s,
                                       ics=ics, h01_norms=h01_norms):
                                vbase = jc * VW
                                pvs = []
                                for i in range(len(ics)):
                                    pvs.append(nc.tensor.matmul(
                                        po0s[i][0:65, :],
                                        v_sb[:, vbase : vbase + 65],
                                        pts[i][:, 0:512],
                                        start=(jc == 0), stop=(jc == 31),
                                    ))
                                for i in range(len(ics)):
                                    pvs.append(nc.tensor.matmul(
                                        po1s[i][0:65, :],
                                        v_sb[:, vbase + 65 : vbase + 130],
                                        pts[i][:, 512:1024],
                                        start=(jc == 0), stop=(jc == 31),
                                    ))
                                if last_qk[0] is not None:
                                    _chain([last_qk[0]] + pvs)
                                else:
                                    _chain(pvs)
                                if jc == 31:
                                    # evacuate po psum fast (frees banks for
                                    # po2); the norm finish is emitted slots
                                    # later so the PE never stalls behind the
                                    # reciprocal chain
                                    evs = []
                                    for i, ic in enumerate(ics):
                                        evs.append(evac_po(po0s[i]) + (ic, 0))
                                    for i, ic in enumerate(ics):
                                        evs.append(evac_po(po1s[i]) + (ic, 1))
                                    h01_norms[0] = evs

                            push_slot(pv_h01)

                        # og loads for the boundary ph3 (AllGathers long done)
                        emit_og_loads(ph3_quarters, ogs)

                        # --- head 2 (row-tiled pair over adjacent key tiles)
                        for t in range(16):
                            # finish deferred h0/h1 norms a few slots into h2,
                            # one per slot so the broadcast + psum-slot cycle
                            # slides into the stream without stalling it
                            if h01_norms[0] and t >= 4:
                                finish_norm(*h01_norms[0].pop(0))
                                if not h01_norms[0]:
                                    h01_norms[0] = None
                            sps, qks = [], []
                            for ic in ics:
                                isl = slice(ic * IC, (ic + 1) * IC)
                                sp = spsum.tile([128, 1024], F32, tag="sp", bufs=2)
                                qks.append(nc.tensor.matmul(
                                    sp[:, 0:512],
                                    kT2[0:64, (2 * t) * 128 : (2 * t + 1) * 128],
                                    qT2[0:64, isl],
                                    start=True, stop=True, tile_position=(0, 0),
                                ))
                                qks.append(nc.tensor.matmul(
                                    sp[:, 512:1024],
                                    kT2[64:128, (2 * t + 1) * 128 : (2 * t + 2) * 128],
                                    qT2[64:128, isl],
                                    start=True, stop=True, tile_position=(64, 0),
                                ))
                                sps.append(sp)
                            _chain(qks)
                            last_qk[0] = qks[-1]
                            pts = exp_batches(sps)

                            def pv_h2(pts=pts, t=t, ics=ics, po2_box=po2_box):
                                # allocated lazily so the buffer handoff from
                                # po0 (shared tag) happens after po0's norm
                                if po2_box[0] is None:
                                    po2_box[0] = [
                                        opsum.tile([128, IC], F32, tag="po0",
                                                   bufs=2, name=f"po2_{ic}")
                                        for ic in ics
                                    ]
                                po2s = po2_box[0]
                                pvs = []
                                for s in range(2):
                                    jc = 2 * t + s
                                    vbase = jc * VW
                                    for i in range(len(ics)):
                                        pvs.append(nc.tensor.matmul(
                                            po2s[i][0:65, :],
                                            v_sb[:, vbase + 130 : vbase + 195],
                                            pts[i][:, s * 512 : (s + 1) * 512],
                                            start=(jc == 0), stop=(jc == 31),
                                        ))
                                if last_qk[0] is not None:
                                    _chain([last_qk[0]] + pvs)
                                else:
                                    _chain(pvs)
                                if t == 15:
                                    # fast evacuation only; finish after the
                                    # boundary ph3 is in the FIFO
                                    po2_box.append([
                                        evac_po(po2s[i]) + (ic, 2)
                                        for i, ic in enumerate(ics)
                                    ])

                            push_slot(pv_h2)
                        while pending:
                            pending.pop(0)()

                        # finish h2 norms and fire the collectives first (the
                        # boundary ph3 matmuls below give the scheduler PE
                        # work to cover the reciprocal-chain latency)
                        for lsb, dat, ic, h in po2_box[1]:
                            finish_norm(lsb, dat, ic, h)
                            nc.gpsimd.collective_compute(
                                "AllGather",
                                mybir.AluOpType.bypass,
                                replica_groups=[[0, 1, 2, 3], [4, 5, 6, 7]],
                                ins=[oT_q[ic][:]],
                                outs=[ag_q[ic][:]],
                            )

                        # boundary output projection fills the group-boundary
                        # PE bubble and keeps HAM warm
                        emit_ph3(ph3_quarters, ogs)

                    # tail: last quarter (waits on the final AllGather)
                    emit_og_loads(TAIL_QUARTERS, ogs)
                    emit_ph3(TAIL_QUARTERS, ogs)

    if dedup:
        removed = _dedup_ldweights(nc)
        print(f"[kernel] dedup_ldweights removed {removed} instructions")
    nc.compile()
    _PROG_CACHE[key] = nc
    return nc


# ---------------------------------------------------------------- host wrapper
def make_in_maps(x, w_qkv, b_qkv, w_out, b_out):
    """Build the 8 per-core input dicts from full inputs."""
    in_maps = []
    import ml_dtypes
    bf16 = ml_dtypes.bfloat16
    xTb = [np.ascontiguousarray(x[b].T.astype(bf16)) for b in range(B)]  # [768, 4096]
    kscale = np.float32(SCALE / 16.0)
    for c in range(N_CORES):
        b = c // 4
        hs = HPC * (c % 4)

        def sect(kind, h):  # q=0,k=1,v=2
            lo = kind * (H * DH) + h * DH
            return w_qkv[:, lo : lo + DH], b_qkv[lo : lo + DH]

        q0, bq0 = sect(0, hs); q1, bq1 = sect(0, hs + 1); q2, bq2 = sect(0, hs + 2)
        k0, bk0 = sect(1, hs); k1, bk1 = sect(1, hs + 1); k2, bk2 = sect(1, hs + 2)
        v0, bv0 = sect(2, hs); v1, bv1 = sect(2, hs + 1); v2, bv2 = sect(2, hs + 2)
        z = np.zeros_like(q2); bz = np.zeros_like(bq2)
        # m-chunks: [v0|v1], [v2|0], [q0|q1], [k0|k1]*s, [q2|k2]*s
        cols = np.concatenate(
            [v0, v1, v2, z, q0, q1, k0 * kscale, k1 * kscale, q2, k2 * kscale],
            axis=1).astype(np.float32)
        bias = np.concatenate(
            [bv0, bv1, bv2, bz, bq0, bq1, bk0 * kscale, bk1 * kscale,
             bq2, bk2 * kscale]).astype(np.float32)
        q = c % 4
        bo = np.zeros((2, 128), np.float32)
        bo[0, :] = b_out[192 * q : 192 * q + 128]
        bo[1, :64] = b_out[192 * q + 128 : 192 * q + 192]
        in_maps.append({
            "xT": xTb[b],
            "wqkv": np.ascontiguousarray(cols.astype(bf16)),
            "bqkv": np.ascontiguousarray(bias.reshape(5, 128)),
            "wout": np.ascontiguousarray(
                w_out[:, 192 * q : 192 * (q + 1)].astype(np.float32)),
            "bout": bo,
        })
    return in_maps


def assemble_output(results):
    out = np.empty((B, N, D), dtype=np.float32)
    for c in range(N_CORES):
        b = c // 4
        q = c % 4
        out[b, :, 192 * q : 192 * (q + 1)] = results[c]["y"].T
    return out


def kernel(x, w_qkv, b_qkv, w_out, b_out):
    from concourse.bass_utils import run_bass_kernel_spmd

    x = np.asarray(x, dtype=np.float32)
    nc = build_program()
    in_maps = make_in_maps(
        x, np.asarray(w_qkv, np.float32), np.asarray(b_qkv, np.float32),
        np.asarray(w_out, np.float32), np.asarray(b_out, np.float32))
    res = run_bass_kernel_spmd(nc, in_maps, core_ids=list(range(N_CORES)))
    return assemble_output(res.results)
